# revision 17
# baseline (speedup 1.0000x reference)
"""Multi-head attention (B=4, S=2048, D=512, H=8) on 8 NeuronCores.

Sharding: core c -> batch b = c//2, head-group g = c%2 (4 heads each).
Each core computes, for its (b, 4-head group):
  qT/kT = (x @ w.T + b).T           [dh-on-partitions layout]
  v     = x @ wv.T                  [natural, + ones column]
  sT    = kT.T-chunks x qT          [j-on-partitions scores, transposed]
  p     = exp(sT/8 + mask_bias_j)   [bf16, mask fused into ACT bias]
  outT/sums via matmul with lhsT=[v | 1]  (sums = row 64)
  attn  = p * (1/sums)  -> DRAM as [h, j, i] (host transposes)
  out   = (outT/sums).T @ woT + (host adds bo, sums partials)

Matmuls run as float32r (tf32-like, full PE rate); p/v are bf16.
"""
import numpy as np
import ml_dtypes

import concourse.bass as bass
import concourse.mybir as mybir
import concourse.tile as tile
from concourse import bacc
from concourse.bass_utils import run_bass_kernel_spmd

F32 = mybir.dt.float32
F32R = mybir.dt.float32r
BF16 = mybir.dt.bfloat16
AF = mybir.ActivationFunctionType
MUL = mybir.AluOpType.mult

B, S, D = 4, 2048, 512
H, DH = 8, 64
HPC = 4            # heads per core
JC = S // 128      # 16 j-chunks
IC = S // 512      # 4 i-chunks of 512
KO = D // 128      # 4 k-chunks for projections
MASK_BIAS = -30000.0

_CACHED_NC = None


def _build():
    nc = bacc.Bacc("TRN2", target_bir_lowering=False, debug=False, num_devices=8)

    # ---- DRAM I/O (per core) ----
    xqT = nc.dram_tensor("xqT", [KO, 128, S], F32R, kind="ExternalInput")
    xkT = nc.dram_tensor("xkT", [KO, 128, S], F32R, kind="ExternalInput")
    xvT = nc.dram_tensor("xvT", [KO, 128, S], F32R, kind="ExternalInput")
    wqT = nc.dram_tensor("wqT", [KO, 128, 256], F32R, kind="ExternalInput")
    wkT = nc.dram_tensor("wkT", [KO, 128, 256], F32R, kind="ExternalInput")
    wvT = nc.dram_tensor("wvT", [KO, 128, 256], F32R, kind="ExternalInput")
    woT = nc.dram_tensor("woT", [2, 128, 512], F32R, kind="ExternalInput")
    bq2 = nc.dram_tensor("bq2", [128, 2], F32, kind="ExternalInput")
    bk2 = nc.dram_tensor("bk2", [128, 2], F32, kind="ExternalInput")
    bv2 = nc.dram_tensor("bv2", [128, 2], F32, kind="ExternalInput")
    maskb = nc.dram_tensor("maskb", [128, JC], F32, kind="ExternalInput")
    onescol = nc.dram_tensor("onescol", [1, 128], F32R, kind="ExternalInput")
    attn_out = nc.dram_tensor("attn", [HPC, S, S], BF16, kind="ExternalOutput")
    po = nc.dram_tensor("po", [S, D], F32, kind="ExternalOutput")

    with tile.TileContext(nc) as tc:
        with tc.tile_pool(name="w", bufs=1) as pw, \
             tc.tile_pool(name="persist", bufs=1) as pp, \
             tc.tile_pool(name="small", bufs=2) as psm:
            # weights / constants
            wq_sb = pw.tile([128, KO, 256], F32R)
            wk_sb = pw.tile([128, KO, 256], F32R)
            wv_sb = pw.tile([128, KO, 256], F32R)
            wo_sb = pw.tile([128, 2, 512], F32R)
            bq_sb = pw.tile([128, 2], F32)
            bk_sb = pw.tile([128, 2], F32)
            bv_sb = pw.tile([128, 2], F32)
            mb_sb = pw.tile([128, JC], F32)
            on_sb = pw.tile([1, 128], F32R)
            for ko in range(KO):
                nc.sync.dma_start(wq_sb[:, ko], wqT[ko])
                nc.sync.dma_start(wk_sb[:, ko], wkT[ko])
                nc.sync.dma_start(wv_sb[:, ko], wvT[ko])
            for dc in range(2):
                nc.sync.dma_start(wo_sb[:, dc], woT[dc])
            nc.sync.dma_start(bq_sb[:], bq2[:])
            nc.sync.dma_start(bk_sb[:], bk2[:])
            nc.sync.dma_start(bv_sb[:], bv2[:])
            nc.sync.dma_start(mb_sb[:], maskb[:])
            nc.sync.dma_start(on_sb[:], onescol[:])

            # persistent activations
            # bf16 so the scores matmuls get FWL (fast weight load) — f32r
            # disables it and the resulting low PE duty keeps HAM cold.
            qT_sb = pp.tile([128, 2, S], BF16)        # [dh-pair, dc, i]
            kT_sb = pp.tile([128, 2, S], BF16)
            vo_sb = [pp.tile([128, JC, 65], BF16, name=f"vo{h}", tag=f"vo{h}")
                     for h in range(HPC)]  # [j, jc, d|1]
            oT_sb = pp.tile([128, 2, S], F32R)        # attn-out transposed [dh, dc, i]

            # ---- projections ----
            with tc.tile_pool(name="x", bufs=1) as px, \
                 tc.tile_pool(name="psp", bufs=4, space="PSUM") as ps_p:
                xq_sb = px.tile([128, KO, S], F32R)
                xk_sb = px.tile([128, KO, S], F32R)
                xv_sb = px.tile([128, KO, S], F32R)
                for ko in range(KO):
                    nc.sync.dma_start(xq_sb[:, ko], xqT[ko])
                    nc.sync.dma_start(xk_sb[:, ko], xkT[ko])
                    nc.sync.dma_start(xv_sb[:, ko], xvT[ko])

                for h in range(HPC):
                    nc.any.memset(vo_sb[h][:, :, 64:65], 1.0)

                # qT/kT: [dh 256 -> 2 chunks of 128p, i]
                for (x_sb, w_sb, b_sb, dst) in ((xq_sb, wq_sb, bq_sb, qT_sb),
                                                (xk_sb, wk_sb, bk_sb, kT_sb)):
                    for dc in range(2):
                        for mc in range(IC):
                            pt = ps_p.tile([128, 512], F32, tag="psp")
                            for ko in range(KO):
                                nc.tensor.matmul(
                                    pt[:], w_sb[:, ko, 128 * dc:128 * dc + 128],
                                    x_sb[:, ko, 512 * mc:512 * mc + 512],
                                    start=(ko == 0), stop=(ko == KO - 1))
                            nc.scalar.activation(
                                dst[:, dc, 512 * mc:512 * mc + 512], pt[:],
                                AF.Identity, bias=b_sb[:, dc:dc + 1])

                # v natural [j, dh]: bv folded in later via sums trick
                for jc in range(JC):
                    pt = ps_p.tile([128, 256], F32, tag="pspv")
                    for ko in range(KO):
                        nc.tensor.matmul(
                            pt[:], xv_sb[:, ko, 128 * jc:128 * jc + 128],
                            wv_sb[:, ko, :],
                            start=(ko == 0), stop=(ko == KO - 1))
                    for h in range(HPC):
                        nc.scalar.copy(vo_sb[h][:, jc, 0:64],
                                       pt[:, 64 * h:64 * h + 64])

            # ---- attention ----
            with tc.tile_pool(name="pT", bufs=31) as ppT, \
                 tc.tile_pool(name="rb", bufs=2) as prb, \
                 tc.tile_pool(name="pss", bufs=2, space="PSUM") as ps_s, \
                 tc.tile_pool(name="pso", bufs=2, space="PSUM") as ps_o, \
                 tc.tile_pool(name="psb", bufs=2, space="PSUM") as ps_b:
                for h in range(HPC):
                    dc, base = h // 2, 64 * (h % 2)
                    p_tiles = []
                    # scores + exp
                    for jc in range(JC):
                        pt_p = ppT.tile([128, S], BF16, tag="pT")
                        for i2 in range(2):
                            st = ps_s.tile([128, 1024], F32, tag="pss")
                            for i4 in range(2):
                                i0 = 1024 * i2 + 512 * i4
                                nc.tensor.matmul(
                                    st[:, 512 * i4:512 * i4 + 512],
                                    kT_sb[base:base + 64, dc, 128 * jc:128 * jc + 128],
                                    qT_sb[base:base + 64, dc, i0:i0 + 512],
                                    start=True, stop=True)
                            nc.scalar.activation(
                                pt_p[:, 1024 * i2:1024 * i2 + 1024], st[:],
                                AF.Exp, bias=mb_sb[:, jc:jc + 1], scale=0.125)
                        p_tiles.append(pt_p)

                    # attn @ [v|1] -> outT (rows 0:64) + sums (row 64)
                    recb = prb.tile([128, S], BF16, tag="rb")
                    for ic in range(IC):
                        ot = ps_o.tile([128, 512], F32, tag="pso")
                        for jc in range(JC):
                            nc.tensor.matmul(
                                ot[0:65, :], vo_sb[h][:, jc, :],
                                p_tiles[jc][:, 512 * ic:512 * ic + 512],
                                start=(jc == 0), stop=(jc == JC - 1))
                        sums = psm.tile([1, 512], F32, tag="sums")
                        rec = psm.tile([1, 512], F32, tag="rec")
                        recr = psm.tile([1, 512], F32R, tag="recr")
                        nc.vector.tensor_copy(sums[:], ot[64:65, :])
                        nc.vector.reciprocal_approx_fast(rec[:], sums[:])
                        nc.vector.tensor_copy(recr[:], rec[:])
                        bt = ps_b.tile([128, 512], F32, tag="psb")
                        nc.tensor.matmul(bt[:], on_sb[:], recr[:],
                                         start=True, stop=True)
                        sl = slice(512 * ic, 512 * ic + 512)
                        nc.vector.tensor_copy(recb[:, sl], bt[:])
                        # outT rows: normalize + bv
                        nc.vector.tensor_tensor(
                            oT_sb[base:base + 64, dc, sl], ot[0:64, :],
                            recb[0:64, sl], MUL)
                        nc.vector.tensor_scalar_add(
                            oT_sb[base:base + 64, dc, sl],
                            oT_sb[base:base + 64, dc, sl],
                            bv_sb[base:base + 64, dc:dc + 1])

                    # normalize p in i-halves (each half only needs its recb
                    # slices, so it overlaps the later attnV ics), then write
                    # attn (transposed layout [j, i]). A slice of the tiles
                    # goes to GpSimd to offload DVE.
                    for half in range(2):
                        hsl = slice(1024 * half, 1024 * half + 1024)
                        for jc in range(JC):
                            if jc % 4 == 3:
                                nc.gpsimd.tensor_mul(
                                    p_tiles[jc][:, hsl], p_tiles[jc][:, hsl],
                                    recb[:, hsl])
                            else:
                                nc.vector.tensor_tensor(
                                    p_tiles[jc][:, hsl], p_tiles[jc][:, hsl],
                                    recb[:, hsl], MUL)
                    for jc in range(JC):
                        nc.sync.dma_start(
                            attn_out[h, 128 * jc:128 * jc + 128, :], p_tiles[jc][:])

                # ---- output projection: po = outT.T @ woT ----
                for mc in range(JC):
                    ot = ps_o.tile([128, 512], F32, tag="pso")
                    for dc in range(2):
                        nc.tensor.matmul(
                            ot[:], oT_sb[:, dc, 128 * mc:128 * mc + 128],
                            wo_sb[:, dc, :], start=(dc == 0), stop=(dc == 1))
                    ob = prb.tile([128, 512], F32, tag="ob")
                    nc.vector.tensor_copy(ob[:], ot[:])
                    nc.sync.dma_start(po[128 * mc:128 * mc + 128, :], ob[:])

    nc.compile()
    return nc


def _prep_core_inputs(query, key, value, attn_mask, wq, bq, wk, bk, wv, bv):
    """Build the 8 per-core input maps (host-side shard + transpose)."""
    f32 = np.float32
    per_g = []
    for g in range(2):
        hs = slice(256 * g, 256 * g + 256)
        per_g.append({
            "wqT": np.ascontiguousarray(wq[hs, :].T.reshape(KO, 128, 256)).astype(f32),
            "wkT": np.ascontiguousarray(wk[hs, :].T.reshape(KO, 128, 256)).astype(f32),
            "wvT": np.ascontiguousarray(wv[hs, :].T.reshape(KO, 128, 256)).astype(f32),
            "woT": None,  # filled by caller (needs wo)
            "bq2": np.ascontiguousarray(bq[hs].reshape(2, 128).T).astype(f32),
            "bk2": np.ascontiguousarray(bk[hs].reshape(2, 128).T).astype(f32),
            "bv2": np.ascontiguousarray(bv[hs].reshape(2, 128).T).astype(f32),
        })
    per_b = []
    for b in range(B):
        per_b.append({
            "xqT": np.ascontiguousarray(query[b].T.reshape(KO, 128, S)).astype(f32),
            "xkT": np.ascontiguousarray(key[b].T.reshape(KO, 128, S)).astype(f32),
            "xvT": np.ascontiguousarray(value[b].T.reshape(KO, 128, S)).astype(f32),
            "maskb": np.ascontiguousarray(
                np.where(attn_mask[b], MASK_BIAS, 0.0).astype(f32)
                .reshape(JC, 128).T),
        })
    ones = np.ones((1, 128), f32)
    in_maps = []
    for c in range(8):
        b, g = c // 2, c % 2
        m = {}
        m.update(per_b[b])
        m.update(per_g[g])
        m["onescol"] = ones
        in_maps.append(m)
    return in_maps


def kernel(query, key, value, attn_mask, wq, bq, wk, bk, wv, bv, wo, bo):
    global _CACHED_NC
    query = np.asarray(query, np.float32)
    key = np.asarray(key, np.float32)
    value = np.asarray(value, np.float32)
    attn_mask = np.asarray(attn_mask, bool)

    in_maps = _prep_core_inputs(np.asarray(query), np.asarray(key),
                                np.asarray(value), attn_mask,
                                np.asarray(wq), np.asarray(bq),
                                np.asarray(wk), np.asarray(bk),
                                np.asarray(wv), np.asarray(bv))
    for c in range(8):
        g = c % 2
        hs = slice(256 * g, 256 * g + 256)
        in_maps[c]["woT"] = np.ascontiguousarray(
            np.asarray(wo)[:, hs].T.reshape(2, 128, 512)).astype(np.float32)

    if _CACHED_NC is None:
        _CACHED_NC = _build()
    res = run_bass_kernel_spmd(_CACHED_NC, in_maps, core_ids=list(range(8))).results

    output = np.empty((B, S, D), np.float32)
    attn = np.empty((B, H, S, S), np.float32)
    bo = np.asarray(bo, np.float32)
    for b in range(B):
        output[b] = res[2 * b]["po"] + res[2 * b + 1]["po"] + bo[None, :]
    for c in range(8):
        b, g = c // 2, c % 2
        a = np.asarray(res[c]["attn"])  # [HPC, j, i] bf16
        for hl in range(HPC):
            attn[b, 4 * g + hl] = a[hl].astype(np.float32).T
    return output, attn


# revision 21
# speedup vs baseline: 1.0757x; 1.0757x over previous
"""Multi-head attention (B=4, S=2048, D=512, H=8) on 8 NeuronCores.

Sharding: core c -> batch b = c//2, head-group g = c%2 (4 heads each).
Each core computes, for its (b, 4-head group):
  qT/kT = (x @ w.T + b).T           [dh-on-partitions layout]
  v     = x @ wv.T                  [natural, + ones column]
  sT    = kT.T-chunks x qT          [j-on-partitions scores, transposed]
  p     = exp(sT/8 + mask_bias_j)   [bf16, mask fused into ACT bias]
  outT/sums via matmul with lhsT=[v | 1]  (sums = row 64)
  attn  = p * (1/sums)  -> DRAM as [h, j, i] (host transposes)
  out   = (outT/sums).T @ woT + (host adds bo, sums partials)

Matmuls run as float32r (tf32-like, full PE rate); p/v are bf16.
"""
import numpy as np
import ml_dtypes

import concourse.bass as bass
import concourse.mybir as mybir
import concourse.tile as tile
from concourse import bacc
from concourse.bass_utils import run_bass_kernel_spmd

F32 = mybir.dt.float32
F32R = mybir.dt.float32r
BF16 = mybir.dt.bfloat16
AF = mybir.ActivationFunctionType
MUL = mybir.AluOpType.mult

B, S, D = 4, 2048, 512
H, DH = 8, 64
HPC = 4            # heads per core
JC = S // 128      # 16 j-chunks
IC = S // 512      # 4 i-chunks of 512
KO = D // 128      # 4 k-chunks for projections
MASK_BIAS = -30000.0

_CACHED_NC = None


def _build():
    nc = bacc.Bacc("TRN2", target_bir_lowering=False, debug=False, num_devices=8)

    # ---- DRAM I/O (per core) ----
    xqT = nc.dram_tensor("xqT", [KO, 128, S], F32R, kind="ExternalInput")
    xkT = nc.dram_tensor("xkT", [KO, 128, S], F32R, kind="ExternalInput")
    xvT = nc.dram_tensor("xvT", [KO, 128, S], F32R, kind="ExternalInput")
    wqT = nc.dram_tensor("wqT", [KO, 128, 256], F32R, kind="ExternalInput")
    wkT = nc.dram_tensor("wkT", [KO, 128, 256], F32R, kind="ExternalInput")
    wvT = nc.dram_tensor("wvT", [KO, 128, 256], F32R, kind="ExternalInput")
    woT = nc.dram_tensor("woT", [2, 128, 512], F32R, kind="ExternalInput")
    bq2 = nc.dram_tensor("bq2", [128, 2], F32, kind="ExternalInput")
    bk2 = nc.dram_tensor("bk2", [128, 2], F32, kind="ExternalInput")
    bv2 = nc.dram_tensor("bv2", [128, 2], F32, kind="ExternalInput")
    maskb = nc.dram_tensor("maskb", [128, JC], F32, kind="ExternalInput")
    onescol = nc.dram_tensor("onescol", [1, 128], F32R, kind="ExternalInput")
    attn_out = nc.dram_tensor("attn", [HPC, S, S], BF16, kind="ExternalOutput")
    po = nc.dram_tensor("po", [S, D], F32, kind="ExternalOutput")

    with tile.TileContext(nc) as tc:
        with tc.tile_pool(name="w", bufs=1) as pw, \
             tc.tile_pool(name="persist", bufs=1) as pp, \
             tc.tile_pool(name="small", bufs=2) as psm:
            # weights / constants
            wq_sb = pw.tile([128, KO, 256], F32R)
            wk_sb = pw.tile([128, KO, 256], F32R)
            wv_sb = pw.tile([128, KO, 256], F32R)
            wo_sb = pw.tile([128, 2, 512], F32R)
            bq_sb = pw.tile([128, 2], F32)
            bk_sb = pw.tile([128, 2], F32)
            bv_sb = pw.tile([128, 2], F32)
            mb_sb = pw.tile([128, JC], F32)
            on_sb = pw.tile([1, 128], F32R)
            for ko in range(KO):
                nc.sync.dma_start(wq_sb[:, ko], wqT[ko])
                nc.sync.dma_start(wk_sb[:, ko], wkT[ko])
                nc.sync.dma_start(wv_sb[:, ko], wvT[ko])
            for dc in range(2):
                nc.sync.dma_start(wo_sb[:, dc], woT[dc])
            nc.sync.dma_start(bq_sb[:], bq2[:])
            nc.sync.dma_start(bk_sb[:], bk2[:])
            nc.sync.dma_start(bv_sb[:], bv2[:])
            nc.sync.dma_start(mb_sb[:], maskb[:])
            nc.sync.dma_start(on_sb[:], onescol[:])

            # persistent activations
            # bf16 so the scores matmuls get FWL (fast weight load) — f32r
            # disables it and the resulting low PE duty keeps HAM cold.
            qT_sb = pp.tile([128, 2, S], BF16)        # [dh-pair, dc, i]
            # kT zero-padded per head to K=128: slot h holds head h's 64 dh
            # rows at their native partitions, zeros elsewhere, so scores
            # matmuls contract over all 128 partitions (keeps HAM warm; the
            # other head's q rows meet zero weights and contribute nothing).
            kTz_sb = pp.tile([128, HPC, S], BF16)
            vo_sb = [pp.tile([128, JC, 65], BF16, name=f"vo{h}", tag=f"vo{h}")
                     for h in range(HPC)]  # [j, jc, d|1]
            oT_sb = pp.tile([128, 2, S], F32R)        # attn-out transposed [dh, dc, i]

            # ---- projections ----
            with tc.tile_pool(name="x", bufs=1) as px, \
                 tc.tile_pool(name="psp", bufs=4, space="PSUM") as ps_p:
                xq_sb = px.tile([128, KO, S], F32R)
                xk_sb = px.tile([128, KO, S], F32R)
                xv_sb = px.tile([128, KO, S], F32R)
                for ko in range(KO):
                    nc.sync.dma_start(xq_sb[:, ko], xqT[ko])
                    nc.sync.dma_start(xk_sb[:, ko], xkT[ko])
                    nc.sync.dma_start(xv_sb[:, ko], xvT[ko])

                for h in range(HPC):
                    nc.any.memset(vo_sb[h][:, :, 64:65], 1.0)
                nc.any.memset(kTz_sb[:], 0.0)

                # qT: [dh 256 -> 2 chunks of 128p, i]
                for dc in range(2):
                    for mc in range(IC):
                        pt = ps_p.tile([128, 512], F32, tag="psp")
                        for ko in range(KO):
                            nc.tensor.matmul(
                                pt[:], wq_sb[:, ko, 128 * dc:128 * dc + 128],
                                xq_sb[:, ko, 512 * mc:512 * mc + 512],
                                start=(ko == 0), stop=(ko == KO - 1))
                        nc.scalar.activation(
                            qT_sb[:, dc, 512 * mc:512 * mc + 512], pt[:],
                            AF.Identity, bias=bq_sb[:, dc:dc + 1])

                # kT, split per head into its zero-padded K=128 slot
                for dc in range(2):
                    for mc in range(IC):
                        pt = ps_p.tile([128, 512], F32, tag="psp")
                        for ko in range(KO):
                            nc.tensor.matmul(
                                pt[:], wk_sb[:, ko, 128 * dc:128 * dc + 128],
                                xk_sb[:, ko, 512 * mc:512 * mc + 512],
                                start=(ko == 0), stop=(ko == KO - 1))
                        for sub in range(2):
                            rs = slice(64 * sub, 64 * sub + 64)
                            nc.scalar.activation(
                                kTz_sb[rs, 2 * dc + sub, 512 * mc:512 * mc + 512],
                                pt[rs, :], AF.Identity,
                                bias=bk_sb[rs, dc:dc + 1])

                # v natural [j, dh]: bv folded in later via sums trick
                for jc in range(JC):
                    pt = ps_p.tile([128, 256], F32, tag="pspv")
                    for ko in range(KO):
                        nc.tensor.matmul(
                            pt[:], xv_sb[:, ko, 128 * jc:128 * jc + 128],
                            wv_sb[:, ko, :],
                            start=(ko == 0), stop=(ko == KO - 1))
                    for h in range(HPC):
                        nc.scalar.copy(vo_sb[h][:, jc, 0:64],
                                       pt[:, 64 * h:64 * h + 64])

            # ---- attention ----
            with tc.tile_pool(name="pT", bufs=29) as ppT, \
                 tc.tile_pool(name="rb", bufs=2) as prb, \
                 tc.tile_pool(name="pss", bufs=2, space="PSUM") as ps_s, \
                 tc.tile_pool(name="pso", bufs=2, space="PSUM") as ps_o, \
                 tc.tile_pool(name="psb", bufs=2, space="PSUM") as ps_b:
                for h in range(HPC):
                    dc, base = h // 2, 64 * (h % 2)
                    p_tiles = []
                    # scores + exp
                    for jc in range(JC):
                        pt_p = ppT.tile([128, S], BF16, tag="pT")
                        for i2 in range(2):
                            st = ps_s.tile([128, 1024], F32, tag="pss")
                            for i4 in range(2):
                                i0 = 1024 * i2 + 512 * i4
                                nc.tensor.matmul(
                                    st[:, 512 * i4:512 * i4 + 512],
                                    kTz_sb[:, h, 128 * jc:128 * jc + 128],
                                    qT_sb[:, dc, i0:i0 + 512],
                                    start=True, stop=True)
                            nc.scalar.activation(
                                pt_p[:, 1024 * i2:1024 * i2 + 1024], st[:],
                                AF.Exp, bias=mb_sb[:, jc:jc + 1], scale=0.125)
                        p_tiles.append(pt_p)

                    # attn @ [v|1] -> outT (rows 0:64) + sums (row 64)
                    recb = prb.tile([128, S], BF16, tag="rb")
                    for ic in range(IC):
                        ot = ps_o.tile([128, 512], F32, tag="pso")
                        for jc in range(JC):
                            nc.tensor.matmul(
                                ot[0:65, :], vo_sb[h][:, jc, :],
                                p_tiles[jc][:, 512 * ic:512 * ic + 512],
                                start=(jc == 0), stop=(jc == JC - 1))
                        sums = psm.tile([1, 512], F32, tag="sums")
                        rec = psm.tile([1, 512], F32, tag="rec")
                        recr = psm.tile([1, 512], F32R, tag="recr")
                        nc.vector.tensor_copy(sums[:], ot[64:65, :])
                        nc.vector.reciprocal_approx_fast(rec[:], sums[:])
                        nc.vector.tensor_copy(recr[:], rec[:])
                        bt = ps_b.tile([128, 512], F32, tag="psb")
                        nc.tensor.matmul(bt[:], on_sb[:], recr[:],
                                         start=True, stop=True)
                        sl = slice(512 * ic, 512 * ic + 512)
                        nc.vector.tensor_copy(recb[:, sl], bt[:])
                        # outT rows: normalize + bv
                        nc.vector.tensor_tensor(
                            oT_sb[base:base + 64, dc, sl], ot[0:64, :],
                            recb[0:64, sl], MUL)
                        nc.vector.tensor_scalar_add(
                            oT_sb[base:base + 64, dc, sl],
                            oT_sb[base:base + 64, dc, sl],
                            bv_sb[base:base + 64, dc:dc + 1])

                    # normalize p in i-halves (each half only needs its recb
                    # slices, so it overlaps the later attnV ics), then write
                    # attn (transposed layout [j, i]). A slice of the tiles
                    # goes to GpSimd to offload DVE.
                    for half in range(2):
                        hsl = slice(1024 * half, 1024 * half + 1024)
                        for jc in range(JC):
                            if jc % 4 == 3:
                                nc.gpsimd.tensor_mul(
                                    p_tiles[jc][:, hsl], p_tiles[jc][:, hsl],
                                    recb[:, hsl])
                            else:
                                nc.vector.tensor_tensor(
                                    p_tiles[jc][:, hsl], p_tiles[jc][:, hsl],
                                    recb[:, hsl], MUL)
                    for jc in range(JC):
                        nc.sync.dma_start(
                            attn_out[h, 128 * jc:128 * jc + 128, :], p_tiles[jc][:])

                # ---- output projection: po = outT.T @ woT ----
                for mc in range(JC):
                    ot = ps_o.tile([128, 512], F32, tag="pso")
                    for dc in range(2):
                        nc.tensor.matmul(
                            ot[:], oT_sb[:, dc, 128 * mc:128 * mc + 128],
                            wo_sb[:, dc, :], start=(dc == 0), stop=(dc == 1))
                    ob = prb.tile([128, 512], F32, tag="ob")
                    nc.vector.tensor_copy(ob[:], ot[:])
                    nc.sync.dma_start(po[128 * mc:128 * mc + 128, :], ob[:])

    nc.compile()
    return nc


def _prep_core_inputs(query, key, value, attn_mask, wq, bq, wk, bk, wv, bv):
    """Build the 8 per-core input maps (host-side shard + transpose)."""
    f32 = np.float32
    per_g = []
    for g in range(2):
        hs = slice(256 * g, 256 * g + 256)
        per_g.append({
            "wqT": np.ascontiguousarray(wq[hs, :].T.reshape(KO, 128, 256)).astype(f32),
            "wkT": np.ascontiguousarray(wk[hs, :].T.reshape(KO, 128, 256)).astype(f32),
            "wvT": np.ascontiguousarray(wv[hs, :].T.reshape(KO, 128, 256)).astype(f32),
            "woT": None,  # filled by caller (needs wo)
            "bq2": np.ascontiguousarray(bq[hs].reshape(2, 128).T).astype(f32),
            "bk2": np.ascontiguousarray(bk[hs].reshape(2, 128).T).astype(f32),
            "bv2": np.ascontiguousarray(bv[hs].reshape(2, 128).T).astype(f32),
        })
    per_b = []
    for b in range(B):
        per_b.append({
            "xqT": np.ascontiguousarray(query[b].T.reshape(KO, 128, S)).astype(f32),
            "xkT": np.ascontiguousarray(key[b].T.reshape(KO, 128, S)).astype(f32),
            "xvT": np.ascontiguousarray(value[b].T.reshape(KO, 128, S)).astype(f32),
            "maskb": np.ascontiguousarray(
                np.where(attn_mask[b], MASK_BIAS, 0.0).astype(f32)
                .reshape(JC, 128).T),
        })
    ones = np.ones((1, 128), f32)
    in_maps = []
    for c in range(8):
        b, g = c // 2, c % 2
        m = {}
        m.update(per_b[b])
        m.update(per_g[g])
        m["onescol"] = ones
        in_maps.append(m)
    return in_maps


def kernel(query, key, value, attn_mask, wq, bq, wk, bk, wv, bv, wo, bo):
    global _CACHED_NC
    query = np.asarray(query, np.float32)
    key = np.asarray(key, np.float32)
    value = np.asarray(value, np.float32)
    attn_mask = np.asarray(attn_mask, bool)

    in_maps = _prep_core_inputs(np.asarray(query), np.asarray(key),
                                np.asarray(value), attn_mask,
                                np.asarray(wq), np.asarray(bq),
                                np.asarray(wk), np.asarray(bk),
                                np.asarray(wv), np.asarray(bv))
    for c in range(8):
        g = c % 2
        hs = slice(256 * g, 256 * g + 256)
        in_maps[c]["woT"] = np.ascontiguousarray(
            np.asarray(wo)[:, hs].T.reshape(2, 128, 512)).astype(np.float32)

    if _CACHED_NC is None:
        _CACHED_NC = _build()
    res = run_bass_kernel_spmd(_CACHED_NC, in_maps, core_ids=list(range(8))).results

    output = np.empty((B, S, D), np.float32)
    attn = np.empty((B, H, S, S), np.float32)
    bo = np.asarray(bo, np.float32)
    for b in range(B):
        output[b] = res[2 * b]["po"] + res[2 * b + 1]["po"] + bo[None, :]
    for c in range(8):
        b, g = c // 2, c % 2
        a = np.asarray(res[c]["attn"])  # [HPC, j, i] bf16
        for hl in range(HPC):
            attn[b, 4 * g + hl] = a[hl].astype(np.float32).T
    return output, attn
